# revision 5
# baseline (speedup 1.0000x reference)
"""Forward-bisect kernel: level-gated cumulative changes from the working
baseline (kernel.py).

level 0: exact baseline structure
level 1: + PV software-pipelined one block behind scores (skew)
level 2: + PV causally narrowed (no pr memsets)
level 3: + wedge mask pre-written to PSUM by PE identity matmul
level 4: + recip row at partition 64 (no rc0 shift DMA)
level 5: + deferred normalization via closure queue (o_sb evict at st end)
level 6: + input/output DMA spread over sync+scalar queues kt-ordered
level 7: + out-projection groups interleaved into pair-3's attention via the
         closure queue (per-supertile odd-head shift DMAs)
level 8: + gpsimd as third input/output DMA queue
level 9: + PV skew 2, normalize chain split into finer closures (recip
         popped a block before broadcast+multiply), projection evictions
         alternating DVE/Act  (on top of level 6; levels 7-8 were
         HW-neutral/negative and are skipped when level >= 9)
level 10: + stage_b (broadcast+multiply) pops gated to kb>=5 so the DVE
         recip chain always completes first; per-supertile odd-head shift
         DMAs; ordered tail flush (recips -> oproj rt 0-11 -> final
         normalizes -> oproj rt 12-15) so the out-projection hides the
         last normalization chains
"""

import numpy as np

B, S, H, D = 4, 2048, 16, 64
DM = H * D          # 1024
NCORES = 8
HPC = H // 2        # 8 heads per core
CQ = HPC * D        # 512 channels per core
NEG = -1.0e9

_PROG_CACHE = {}
LEVEL = 10


def build_program(rep_qkv=1, rep_attn=1, rep_oproj=1, rep_all=1, level=None):
    import concourse.mybir as mybir
    import concourse.tile as tile
    from concourse import bacc

    if level is None:
        level = LEVEL
    dt = mybir.dt
    f32 = dt.float32
    bf16 = dt.bfloat16
    AF = mybir.ActivationFunctionType
    MULT = mybir.AluOpType.mult

    nc = bacc.Bacc(None)
    xt = nc.declare_dram_parameter("xt", [DM, S], bf16, isOutput=False)
    wqk = nc.declare_dram_parameter("wqk", [DM, 2 * CQ], bf16, isOutput=False)
    wv = nc.declare_dram_parameter("wv", [DM, CQ], bf16, isOutput=False)
    wo = nc.declare_dram_parameter("wo", [CQ, DM], bf16, isOutput=False)
    mask = nc.declare_dram_parameter("mask", [128, 128], f32, isOutput=False)
    maskb = nc.declare_dram_parameter("maskb", [128, 128], bf16, isOutput=False)
    eye = nc.declare_dram_parameter("eye", [128, 128], bf16, isOutput=False)
    out = nc.declare_dram_parameter("out", [S, DM], f32, isOutput=True)

    KT = DM // 128
    NRT = S // 128
    NRC = S // 512
    NP = HPC // 2
    NST = S // 512

    with tile.TileContext(nc) as tc:
        with (
            tc.tile_pool(name="persist", bufs=1) as pp,
            tc.tile_pool(name="probs", bufs=4) as probsp,
            tc.tile_pool(name="recip", bufs=4) as recipp,
            tc.tile_pool(name="ostage", bufs=3) as ostagep,
            tc.tile_pool(name="psmm", bufs=2, space="PSUM") as psmm,
            tc.tile_pool(name="pssc", bufs=2, space="PSUM") as pssc,
            tc.tile_pool(name="psout", bufs=2, space="PSUM") as psout,
        ):
            # ---- load inputs to SBUF ----
            if level >= 6:
                queues = [nc.sync, nc.scalar, nc.gpsimd][:3 if level == 8 else 2]
                qcnt = [0]

                def load(t, src):
                    queues[qcnt[0] % len(queues)].dma_start(out=t[:], in_=src)
                    qcnt[0] += 1
            else:
                def load(t, src, q=[0]):
                    (nc.sync if q[0] % 3 == 0 else nc.scalar).dma_start(
                        out=t[:], in_=src)
                    q[0] += 1

            mask_sb = pp.tile([128, 128], f32, tag="mask", name="mask")
            load(mask_sb, mask[:, :])
            maskb_sb = pp.tile([128, 128], bf16, tag="maskb", name="maskb")
            load(maskb_sb, maskb[:, :])
            eye_sb = pp.tile([128, 128], bf16, tag="eye", name="eye")
            load(eye_sb, eye[:, :])
            xt_sb = []
            wqk_sb = []
            wv_sb = []
            for i in range(KT):
                t = pp.tile([128, S], bf16, tag=f"xt{i}", name=f"xt{i}")
                load(t, xt[128 * i:128 * (i + 1), :])
                xt_sb.append(t)
                t = pp.tile([128, 2 * CQ], bf16, tag=f"wqk{i}", name=f"wqk{i}")
                load(t, wqk[128 * i:128 * (i + 1), :])
                wqk_sb.append(t)
            for i in range(KT):
                t = pp.tile([128, CQ], bf16, tag=f"wv{i}", name=f"wv{i}")
                load(t, wv[128 * i:128 * (i + 1), :])
                wv_sb.append(t)
            wo_sb = []
            for c in range(CQ // 128):
                t = pp.tile([128, DM], bf16, tag=f"wo{c}", name=f"wo{c}")
                load(t, wo[128 * c:128 * (c + 1), :])
                wo_sb.append(t)
            ones_sb = pp.tile([128, 64], bf16, tag="ones", name="ones")
            nc.vector.memset(ones_sb[:, :], 1.0)

            qT = [pp.tile([128, S], bf16, tag=f"qT{p}", name=f"qT{p}") for p in range(NP)]
            kT = [pp.tile([128, S], bf16, tag=f"kT{p}", name=f"kT{p}") for p in range(NP)]
            v_rm = [pp.tile([128, HPC * 65], bf16, tag=f"v{rt}", name=f"v{rt}") for rt in range(NRT)]
            oT = [pp.tile([128, S], bf16, tag=f"oT{p}", name=f"oT{p}") for p in range(NP)]

            def v_group(rt):
                v_view = v_rm[rt].rearrange("p (h c) -> p h c", c=65)
                nc.vector.memset(v_view[:, :, 64:65], 1.0)
                ps = psmm.tile([128, 512], f32, tag="mm", name="mm")
                for kt in range(KT):
                    nc.tensor.matmul(
                        ps[:],
                        lhsT=xt_sb[kt][:, 128 * rt:128 * (rt + 1)],
                        rhs=wv_sb[kt][:],
                        start=(kt == 0),
                        stop=(kt == KT - 1),
                    )
                if level >= 9 and rt % 2 == 1:
                    nc.scalar.copy(
                        v_view[:, :, 0:64], ps.rearrange("p (h c) -> p h c", c=64)
                    )
                else:
                    nc.vector.tensor_copy(
                        v_view[:, :, 0:64], ps.rearrange("p (h c) -> p h c", c=64)
                    )

            for _arep in range(rep_all):

             def qk_group(p, ct, rc):
                 dst = qT[p] if ct < NP else kT[p]
                 ps = psmm.tile([128, 512], f32, tag="mm", name="mm")
                 for kt in range(KT):
                     nc.tensor.matmul(
                         ps[:],
                         lhsT=wqk_sb[kt][:, 128 * ct:128 * (ct + 1)],
                         rhs=xt_sb[kt][:, 512 * rc:512 * (rc + 1)],
                         start=(kt == 0),
                         stop=(kt == KT - 1),
                     )
                 if level >= 9 and rc % 2 == 1:
                     nc.scalar.copy(dst[:, 512 * rc:512 * (rc + 1)], ps[:])
                 else:
                     nc.vector.tensor_copy(dst[:, 512 * rc:512 * (rc + 1)], ps[:])

             qk_groups = lambda p: [
                 (lambda a=p, b=ct, c=rc: qk_group(a, b, c))
                 for ct in (p, NP + p) for rc in range(NRC)
             ]

             for _qrep in range(rep_qkv):
                 for g in qk_groups(0):
                     g()
             for rt in range(4):
                 v_group(rt)

             normq = []

             for p in range(NP):
                 pending = []
                 if p == 0:
                     pending += [lambda a=rt: v_group(a) for rt in range(4, NRT)]
                 pending += qk_groups(p + 1) if p + 1 < NP else []
                 if rep_qkv > 1:
                     for _qrep in range(rep_qkv - 1):
                         for g in qk_groups(p):
                             g()
                     if p == 0:
                         for rt in range(4, NRT):
                             v_group(rt)
                         pending = qk_groups(p + 1) if p + 1 < NP else []

                 hi_sb = recipp.tile([64, S], bf16, tag="hi", name="hi")

                 def norm_recip(osb):
                     rc_sb = recipp.tile([128, 512], bf16, tag="recip", name="recip")
                     with nc.allow_low_precision(reason="bf16 softmax denom"):
                         nc.vector.reciprocal(rc_sb[64:65, :], osb[64:65, :])
                     return rc_sb

                 def norm_apply(st, hh, osb, rc_sb, p=p, hi_sb=hi_sb):
                     bc_ps = psmm.tile([64, 512], f32, tag="mm", name="mm")
                     nc.tensor.matmul(
                         bc_ps[:, :],
                         lhsT=ones_sb[64:65, 0:64],
                         rhs=rc_sb[64:65, :],
                         start=True,
                         stop=True,
                         tile_position=(64, 0),
                     )
                     dst = (
                         oT[p][0:64, 512 * st:512 * (st + 1)]
                         if hh == 0
                         else hi_sb[:, 512 * st:512 * (st + 1)]
                     )
                     nc.vector.scalar_tensor_tensor(
                         dst, osb[0:64, :], 1.0, bc_ps[:, :], MULT, MULT
                     )
                     if level >= 10 and hh == 1:
                         nc.sync.dma_start(
                             out=oT[p][64:128, 512 * st:512 * (st + 1)],
                             in_=hi_sb[:, 512 * st:512 * (st + 1)],
                         )

                 def normalize(st, hh, osb, p=p, hi_sb=hi_sb):
                     # osb: [65,512] SBUF f32 (level>=5) or PSUM tile (level<5)
                     rc_sb = recipp.tile([128, 512], bf16, tag="recip", name="recip")
                     with nc.allow_low_precision(reason="bf16 softmax denom"):
                         nc.vector.reciprocal(rc_sb[64:65, :], osb[64:65, :])
                     bc_ps = psmm.tile([64, 512], f32, tag="mm", name="mm")
                     if level >= 4:
                         nc.tensor.matmul(
                             bc_ps[:, :],
                             lhsT=ones_sb[64:65, 0:64],
                             rhs=rc_sb[64:65, :],
                             start=True,
                             stop=True,
                             tile_position=(64, 0),
                         )
                     else:
                         rc0_sb = recipp.tile([1, 512], bf16, tag="recip0", name="recip0")
                         nc.sync.dma_start(out=rc0_sb[0:1, :], in_=rc_sb[64:65, :])
                         nc.tensor.matmul(
                             bc_ps[:, :],
                             lhsT=ones_sb[0:1, 0:64],
                             rhs=rc0_sb[0:1, :],
                             start=True,
                             stop=True,
                         )
                     dst = (
                         oT[p][0:64, 512 * st:512 * (st + 1)]
                         if hh == 0
                         else hi_sb[:, 512 * st:512 * (st + 1)]
                     )
                     nc.vector.scalar_tensor_tensor(
                         dst, osb[0:64, :], 1.0, bc_ps[:, :], MULT, MULT
                     )
                     if level == 7 and hh == 1:
                         # per-supertile odd-head partition shift
                         nc.sync.dma_start(
                             out=oT[p][64:128, 512 * st:512 * (st + 1)],
                             in_=hi_sb[:, 512 * st:512 * (st + 1)],
                         )

                 def oproj_group(rt):
                     st_sb = ostagep.tile([128, 1024], f32, tag="ostage", name="ostage")
                     for o2 in range(2):
                         ps = psmm.tile([128, 512], f32, tag="mm", name="mm")
                         for c in range(4):
                             nc.tensor.matmul(
                                 ps[:],
                                 lhsT=oT[c][:, 128 * rt:128 * (rt + 1)],
                                 rhs=wo_sb[c][:, 512 * o2:512 * (o2 + 1)],
                                 start=(c == 0),
                                 stop=(c == 3),
                             )
                         if o2 == 0:
                             nc.vector.tensor_copy(
                                 st_sb[:, 512 * o2:512 * (o2 + 1)], ps[:]
                             )
                         else:
                             nc.scalar.copy(
                                 st_sb[:, 512 * o2:512 * (o2 + 1)], ps[:]
                             )
                     ([nc.sync, nc.scalar][rt % 2] if level >= 6 else nc.sync).dma_start(
                         out=out[128 * rt:128 * (rt + 1), :], in_=st_sb[:]
                     )

                 st_seen = {}
                 fill_iter = iter(pending)
                 nblocks = sum(4 * s + 4 for s in range(NST))
                 stride = max(1, nblocks // max(len(pending), 1))
                 blk = 0
                 for st in [s for s in range(NST) for _ in range(rep_attn)]:
                     out_ps = [
                         psout.tile([65, 512], f32, tag="o", name="o")
                         for _ in range(2)
                     ]
                     nkb = 4 * st + 4

                     def pv_full(kb, pr, first, out_ps=out_ps, nkb=nkb, p=p):
                         for hh in range(2):
                             base = 512 * hh
                             h = 2 * p + hh
                             nc.tensor.matmul(
                                 out_ps[hh][:, :],
                                 lhsT=v_rm[kb][:, 65 * h:65 * h + 65],
                                 rhs=pr[:, base:base + 512],
                                 start=first,
                                 stop=(kb == nkb - 1),
                             )

                     def pv_narrow(kb, r, qi0, pr, first, out_ps=out_ps, p=p):
                         for hh in range(2):
                             base = 512 * hh
                             h = 2 * p + hh
                             lhs = v_rm[kb][:, 65 * h:65 * h + 65]
                             if r < 0:
                                 nc.tensor.matmul(
                                     out_ps[hh][:, :],
                                     lhsT=lhs,
                                     rhs=pr[:, base:base + 512],
                                     start=first,
                                     stop=False,
                                     skip_group_check=True,
                                 )
                             else:
                                 nc.tensor.matmul(
                                     out_ps[hh][:, qi0:qi0 + 128],
                                     lhsT=lhs,
                                     rhs=pr[:, base + qi0:base + qi0 + 128],
                                     start=first,
                                     stop=True,
                                     skip_group_check=True,
                                 )
                                 if r < 3:
                                     nc.tensor.matmul(
                                         out_ps[hh][:, qi0 + 128:512],
                                         lhsT=lhs,
                                         rhs=pr[:, base + qi0 + 128:base + 512],
                                         start=first,
                                         stop=False,
                                         skip_group_check=True,
                                     )

                     pv_q = []
                     skew = 2 if level >= 9 else 1
                     for kb in range(nkb):
                         r = kb - 4 * st
                         qi0 = 128 * r if r > 0 else 0
                         first, last = (kb == 0), (kb == nkb - 1)
                         sc = pssc.tile([128, 1024], f32, tag="sc", name="sc")
                         if r >= 0 and level >= 3:
                             for hh in range(2):
                                 base = 512 * hh
                                 nc.tensor.matmul(
                                     sc[:, base + qi0:base + qi0 + 128],
                                     lhsT=eye_sb[:, :],
                                     rhs=maskb_sb[:, :],
                                     start=True,
                                     stop=False,
                                     skip_group_check=True,
                                 )
                         for hh in range(2):
                             base, lo = 512 * hh, 64 * hh
                             if r >= 0 and level >= 3:
                                 nc.tensor.matmul(
                                     sc[:, base + qi0:base + qi0 + 128],
                                     lhsT=kT[p][lo:lo + 64, 128 * kb:128 * (kb + 1)],
                                     rhs=qT[p][lo:lo + 64, 512 * st + qi0:512 * st + qi0 + 128],
                                     start=False,
                                     stop=True,
                                     tile_position=(lo, 0),
                                     skip_group_check=True,
                                 )
                                 if r < 3:
                                     nc.tensor.matmul(
                                         sc[:, base + qi0 + 128:base + 512],
                                         lhsT=kT[p][lo:lo + 64, 128 * kb:128 * (kb + 1)],
                                         rhs=qT[p][lo:lo + 64, 512 * st + qi0 + 128:512 * (st + 1)],
                                         start=True,
                                         stop=True,
                                         tile_position=(lo, 0),
                                         skip_group_check=True,
                                     )
                             else:
                                 nc.tensor.matmul(
                                     sc[:, base + qi0:base + 512],
                                     lhsT=kT[p][lo:lo + 64, 128 * kb:128 * (kb + 1)],
                                     rhs=qT[p][lo:lo + 64, 512 * st + qi0:512 * (st + 1)],
                                     start=True,
                                     stop=True,
                                     tile_position=(lo, 0),
                                 )
                         if r >= 0 and level < 3:
                             for hh in range(2):
                                 base = 512 * hh
                                 nc.vector.tensor_add(
                                     sc[:, base + qi0:base + qi0 + 128],
                                     sc[:, base + qi0:base + qi0 + 128],
                                     mask_sb[:, :],
                                 )
                         pr = probsp.tile([128, 1024], bf16, tag="pr", name="pr")
                         if qi0 == 0:
                             nc.scalar.activation(pr[:], sc[:], AF.Exp)
                         else:
                             pr_v = pr.rearrange("p (h q) -> p h q", h=2)
                             sc_v = sc.rearrange("p (h q) -> p h q", h=2)
                             if level < 2:
                                 nc.vector.memset(pr_v[:, :, 0:qi0], 0.0)
                             nc.scalar.activation(
                                 pr_v[:, :, qi0:512], sc_v[:, :, qi0:512], AF.Exp
                             )

                         if level >= 2:
                             this_pv = (
                                 lambda f=pv_narrow, kb=kb, r=r, qi0=qi0, pr=pr,
                                 first=first: f(kb, r, qi0, pr, first)
                             )
                         else:
                             this_pv = (
                                 lambda f=pv_full, kb=kb, pr=pr, first=first:
                                 f(kb, pr, first)
                             )
                         if level >= 1:
                             pv_q.append(this_pv)
                             if len(pv_q) > skew:
                                 pv_q.pop(0)()
                         else:
                             this_pv()

                         if level >= 10:
                             if normq and kb >= normq[0][0]:
                                 normq.pop(0)[2]()
                         else:
                             if level >= 9:
                                 popper = kb >= 1
                             elif level == 7 or level == 8:
                                 popper = kb % 2 == 1 or p == NP - 1
                             else:
                                 popper = kb % 2 == 1
                             if level >= 5 and popper and normq:
                                 normq.pop(0)()
                         blk += 1
                         if blk % stride == 0:
                             g = next(fill_iter, None)
                             if g is not None:
                                 g()
                     if level >= 1:
                         for g in pv_q:
                             g()
                         pv_q = []
                     if level >= 9:
                         stage_bs = []
                         for hh in range(2):
                             o_sb = recipp.tile([65, 512], f32, tag="osb", name="osb")
                             nc.vector.tensor_copy(o_sb[:, :], out_ps[hh][:, :])
                             holder = {}

                             def stage_a(f=norm_recip, osb=o_sb, holder=holder):
                                 holder["rc"] = f(osb)

                             def stage_b(f=norm_apply, st=st, hh=hh, osb=o_sb,
                                         holder=holder):
                                 f(st, hh, osb, holder["rc"])

                             if level >= 10:
                                 normq.append((1, "a", stage_a))
                                 stage_bs.append((5, "b", stage_b))
                             else:
                                 normq.append(stage_a)
                                 stage_bs.append(stage_b)
                         normq.extend(stage_bs)
                     elif level >= 5:
                         for hh in range(2):
                             o_sb = recipp.tile([65, 512], f32, tag="osb", name="osb")
                             nc.vector.tensor_copy(o_sb[:, :], out_ps[hh][:, :])
                             normq.append(
                                 lambda f=normalize, st=st, hh=hh, osb=o_sb:
                                 f(st, hh, osb)
                             )
                         st_seen[st] = st_seen.get(st, 0) + 1
                         if (level in (7, 8) and p == NP - 1
                                 and st_seen[st] == rep_attn):
                             for rt in range(4 * st, 4 * st + 4):
                                 normq.append(
                                     lambda f=oproj_group, rt=rt: f(rt)
                                 )
                     else:
                         for hh in range(2):
                             o_sb = recipp.tile([65, 512], f32, tag="osb", name="osb")
                             nc.vector.tensor_copy(o_sb[:, :], out_ps[hh][:, :])
                             normalize(st, hh, o_sb)
                 for g in fill_iter:
                     g()
                 if level in (7, 8) or level >= 10:
                     pass  # per-supertile shifts emitted inside normalize
                 elif level >= 5:
                     def shift(hi_sb=hi_sb, p=p):
                         nc.sync.dma_start(out=oT[p][64:128, :], in_=hi_sb[:, :])
                     normq.append(shift)
                 else:
                     nc.sync.dma_start(out=oT[p][64:128, :], in_=hi_sb[:, :])

             def oproj_rt(rt):
                 st_sb = ostagep.tile([128, 1024], f32, tag="ostage", name="ostage")
                 for o2 in range(2):
                     ps = psmm.tile([128, 512], f32, tag="mm", name="mm")
                     for c in range(4):
                         nc.tensor.matmul(
                             ps[:],
                             lhsT=oT[c][:, 128 * rt:128 * (rt + 1)],
                             rhs=wo_sb[c][:, 512 * o2:512 * (o2 + 1)],
                             start=(c == 0),
                             stop=(c == 3),
                         )
                     if o2 == 0:
                         nc.vector.tensor_copy(
                             st_sb[:, 512 * o2:512 * (o2 + 1)], ps[:]
                         )
                     else:
                         nc.scalar.copy(
                             st_sb[:, 512 * o2:512 * (o2 + 1)], ps[:]
                         )
                 ([nc.sync, nc.scalar][rt % 2] if level >= 6 else nc.sync).dma_start(
                     out=out[128 * rt:128 * (rt + 1), :], in_=st_sb[:]
                 )

             if level >= 10:
                 # tail: start recips, hide the final normalize chains under
                 # oproj rows that only need supertiles 0-2, finish with the
                 # last supertile's rows
                 while normq and normq[0][1] == "a":
                     normq.pop(0)[2]()
                 for rt in range(12):
                     oproj_rt(rt)
                 while normq:
                     normq.pop(0)[2]()
                 for rt in range(12, NRT):
                     oproj_rt(rt)
                 for _orep in range(rep_oproj - 1):
                     for rt in range(NRT):
                         oproj_rt(rt)
             else:
                 while normq:
                     normq.pop(0)()
                 for _orep in range(rep_oproj - 1 if level in (7, 8) else rep_oproj):
                     for rt in range(NRT):
                         oproj_rt(rt)
    nc.finalize()
    return nc


def get_program():
    if "nc" not in _PROG_CACHE:
        _PROG_CACHE["nc"] = build_program()
    return _PROG_CACHE["nc"]


def make_in_maps(x, w_qkv, w_out):
    import ml_dtypes

    bf = ml_dtypes.bfloat16
    x = np.asarray(x, dtype=np.float32)
    w_qkv = np.asarray(w_qkv, dtype=np.float32)
    w_out = np.asarray(w_out, dtype=np.float32)
    scale = float(D) ** -0.5
    p_idx = np.arange(128)[:, None]
    j_idx = np.arange(128)[None, :]
    maskf = np.where(p_idx > j_idx, NEG, 0.0).astype(np.float32)
    in_maps = []
    for c in range(NCORES):
        b, hh = c // 2, c % 2
        q0 = CQ * hh
        wq = (w_qkv[:, q0:q0 + CQ] * scale).astype(bf)
        wk = w_qkv[:, DM + q0:DM + q0 + CQ].astype(bf)
        in_maps.append(
            {
                "xt": np.ascontiguousarray(x[b].T).astype(bf),
                "wqk": np.concatenate([wq, wk], axis=1),
                "wv": w_qkv[:, 2 * DM + q0:2 * DM + q0 + CQ].astype(bf),
                "wo": w_out[q0:q0 + CQ, :].astype(bf),
                "mask": maskf,
                "maskb": maskf.astype(bf),
                "eye": np.eye(128, dtype=bf),
            }
        )
    return in_maps


def gather(results):
    outs = [np.asarray(results[c]["out"], dtype=np.float32) for c in range(NCORES)]
    return np.stack([outs[2 * b] + outs[2 * b + 1] for b in range(B)], axis=0)


def kernel(x, w_qkv, w_out):
    from concourse.bass_utils import run_bass_kernel_spmd

    nc = get_program()
    in_maps = make_in_maps(x, w_qkv, w_out)
    res = run_bass_kernel_spmd(nc, in_maps, list(range(NCORES)))
    return gather(res.results)


# revision 6
# speedup vs baseline: 1.0628x; 1.0628x over previous
"""Trainium2 Bass kernel for causal MultiHeadAttention.

Problem: B=4, S=2048, H=16, D=64, DM=1024, fp32 I/O.
  qkv = x @ w_qkv ; causal softmax attention per head ; out = attn @ w_out

Sharding (8 cores): 4-way batch x 2-way heads. Core c handles batch c//2 and
heads (c%2)*8 .. +8; each core computes a partial out-projection (its 512
attention channels x full w_out row-slice) and the host sums the two
head-half partials per batch while unsharding.

Per-core dataflow (bf16 matmul inputs, fp32 PSUM): staged Q/K/V projections
feed a block-causal attention stream (key blocks of 128 x query supertiles
of 512, two heads packed in the PE array), normalization runs as deferred
closures inside the next supertile's block stream, and the out-projection
tail hides the final normalize chains. The optimization history is encoded
as cumulative levels (LEVEL=10 enables the best verified set):

level 0: session-1 baseline structure
level 1: + PV software-pipelined one block behind scores (skew)
level 2: + PV causally narrowed (no pr memsets)
level 3: + wedge mask pre-written to PSUM by PE identity matmul
level 4: + recip row at partition 64 (no rc0 shift DMA)
level 5: + deferred normalization via closure queue (o_sb evict at st end)
level 6: + input/output DMA spread over sync+scalar queues kt-ordered
level 7: + out-projection groups interleaved into pair-3's attention via the
         closure queue (per-supertile odd-head shift DMAs)
level 8: + gpsimd as third input/output DMA queue
level 9: + PV skew 2, normalize chain split into finer closures (recip
         popped a block before broadcast+multiply), projection evictions
         alternating DVE/Act  (on top of level 6; levels 7-8 were
         HW-neutral/negative and are skipped when level >= 9)
level 10: + stage_b (broadcast+multiply) pops gated to kb>=5 so the DVE
         recip chain always completes first; per-supertile odd-head shift
         DMAs; ordered tail flush (recips -> oproj rt 0-11 -> final
         normalizes -> oproj rt 12-15) so the out-projection hides the
         last normalization chains
"""

import numpy as np

B, S, H, D = 4, 2048, 16, 64
DM = H * D          # 1024
NCORES = 8
HPC = H // 2        # 8 heads per core
CQ = HPC * D        # 512 channels per core
NEG = -1.0e9

_PROG_CACHE = {}
LEVEL = 10


def build_program(rep_qkv=1, rep_attn=1, rep_oproj=1, rep_all=1, level=None):
    import concourse.mybir as mybir
    import concourse.tile as tile
    from concourse import bacc

    if level is None:
        level = LEVEL
    dt = mybir.dt
    f32 = dt.float32
    bf16 = dt.bfloat16
    AF = mybir.ActivationFunctionType
    MULT = mybir.AluOpType.mult

    nc = bacc.Bacc(None)
    xt = nc.declare_dram_parameter("xt", [DM, S], bf16, isOutput=False)
    wqk = nc.declare_dram_parameter("wqk", [DM, 2 * CQ], bf16, isOutput=False)
    wv = nc.declare_dram_parameter("wv", [DM, CQ], bf16, isOutput=False)
    wo = nc.declare_dram_parameter("wo", [CQ, DM], bf16, isOutput=False)
    mask = nc.declare_dram_parameter("mask", [128, 128], f32, isOutput=False)
    maskb = nc.declare_dram_parameter("maskb", [128, 128], bf16, isOutput=False)
    eye = nc.declare_dram_parameter("eye", [128, 128], bf16, isOutput=False)
    out = nc.declare_dram_parameter("out", [S, DM], f32, isOutput=True)

    KT = DM // 128
    NRT = S // 128
    NRC = S // 512
    NP = HPC // 2
    NST = S // 512

    with tile.TileContext(nc) as tc:
        with (
            tc.tile_pool(name="persist", bufs=1) as pp,
            tc.tile_pool(name="probs", bufs=4) as probsp,
            tc.tile_pool(name="recip", bufs=4) as recipp,
            tc.tile_pool(name="ostage", bufs=3) as ostagep,
            tc.tile_pool(name="psmm", bufs=2, space="PSUM") as psmm,
            tc.tile_pool(name="pssc", bufs=2, space="PSUM") as pssc,
            tc.tile_pool(name="psout", bufs=2, space="PSUM") as psout,
        ):
            # ---- load inputs to SBUF ----
            if level >= 6:
                queues = [nc.sync, nc.scalar, nc.gpsimd][:3 if level == 8 else 2]
                qcnt = [0]

                def load(t, src):
                    queues[qcnt[0] % len(queues)].dma_start(out=t[:], in_=src)
                    qcnt[0] += 1
            else:
                def load(t, src, q=[0]):
                    (nc.sync if q[0] % 3 == 0 else nc.scalar).dma_start(
                        out=t[:], in_=src)
                    q[0] += 1

            mask_sb = pp.tile([128, 128], f32, tag="mask", name="mask")
            load(mask_sb, mask[:, :])
            maskb_sb = pp.tile([128, 128], bf16, tag="maskb", name="maskb")
            load(maskb_sb, maskb[:, :])
            eye_sb = pp.tile([128, 128], bf16, tag="eye", name="eye")
            load(eye_sb, eye[:, :])
            xt_sb = []
            wqk_sb = []
            wv_sb = []
            for i in range(KT):
                t = pp.tile([128, S], bf16, tag=f"xt{i}", name=f"xt{i}")
                load(t, xt[128 * i:128 * (i + 1), :])
                xt_sb.append(t)
                t = pp.tile([128, 2 * CQ], bf16, tag=f"wqk{i}", name=f"wqk{i}")
                load(t, wqk[128 * i:128 * (i + 1), :])
                wqk_sb.append(t)
            for i in range(KT):
                t = pp.tile([128, CQ], bf16, tag=f"wv{i}", name=f"wv{i}")
                load(t, wv[128 * i:128 * (i + 1), :])
                wv_sb.append(t)
            wo_sb = []
            for c in range(CQ // 128):
                t = pp.tile([128, DM], bf16, tag=f"wo{c}", name=f"wo{c}")
                load(t, wo[128 * c:128 * (c + 1), :])
                wo_sb.append(t)
            ones_sb = pp.tile([128, 64], bf16, tag="ones", name="ones")
            nc.vector.memset(ones_sb[:, :], 1.0)

            qT = [pp.tile([128, S], bf16, tag=f"qT{p}", name=f"qT{p}") for p in range(NP)]
            kT = [pp.tile([128, S], bf16, tag=f"kT{p}", name=f"kT{p}") for p in range(NP)]
            v_rm = [pp.tile([128, HPC * 65], bf16, tag=f"v{rt}", name=f"v{rt}") for rt in range(NRT)]
            oT = [pp.tile([128, S], bf16, tag=f"oT{p}", name=f"oT{p}") for p in range(NP)]

            def v_group(rt):
                v_view = v_rm[rt].rearrange("p (h c) -> p h c", c=65)
                nc.vector.memset(v_view[:, :, 64:65], 1.0)
                ps = psmm.tile([128, 512], f32, tag="mm", name="mm")
                for kt in range(KT):
                    nc.tensor.matmul(
                        ps[:],
                        lhsT=xt_sb[kt][:, 128 * rt:128 * (rt + 1)],
                        rhs=wv_sb[kt][:],
                        start=(kt == 0),
                        stop=(kt == KT - 1),
                    )
                if level >= 9 and rt % 2 == 1:
                    nc.scalar.copy(
                        v_view[:, :, 0:64], ps.rearrange("p (h c) -> p h c", c=64)
                    )
                else:
                    nc.vector.tensor_copy(
                        v_view[:, :, 0:64], ps.rearrange("p (h c) -> p h c", c=64)
                    )

            for _arep in range(rep_all):

             def qk_group(p, ct, rc):
                 dst = qT[p] if ct < NP else kT[p]
                 ps = psmm.tile([128, 512], f32, tag="mm", name="mm")
                 for kt in range(KT):
                     nc.tensor.matmul(
                         ps[:],
                         lhsT=wqk_sb[kt][:, 128 * ct:128 * (ct + 1)],
                         rhs=xt_sb[kt][:, 512 * rc:512 * (rc + 1)],
                         start=(kt == 0),
                         stop=(kt == KT - 1),
                     )
                 if level >= 9 and rc % 2 == 1:
                     nc.scalar.copy(dst[:, 512 * rc:512 * (rc + 1)], ps[:])
                 else:
                     nc.vector.tensor_copy(dst[:, 512 * rc:512 * (rc + 1)], ps[:])

             qk_groups = lambda p: [
                 (lambda a=p, b=ct, c=rc: qk_group(a, b, c))
                 for ct in (p, NP + p) for rc in range(NRC)
             ]

             for _qrep in range(rep_qkv):
                 for g in qk_groups(0):
                     g()
             for rt in range(4):
                 v_group(rt)

             normq = []

             for p in range(NP):
                 pending = []
                 if p == 0:
                     pending += [lambda a=rt: v_group(a) for rt in range(4, NRT)]
                 pending += qk_groups(p + 1) if p + 1 < NP else []
                 if rep_qkv > 1:
                     for _qrep in range(rep_qkv - 1):
                         for g in qk_groups(p):
                             g()
                     if p == 0:
                         for rt in range(4, NRT):
                             v_group(rt)
                         pending = qk_groups(p + 1) if p + 1 < NP else []

                 hi_sb = recipp.tile([64, S], bf16, tag="hi", name="hi")

                 def norm_recip(osb):
                     rc_sb = recipp.tile([128, 512], bf16, tag="recip", name="recip")
                     with nc.allow_low_precision(reason="bf16 softmax denom"):
                         nc.vector.reciprocal(rc_sb[64:65, :], osb[64:65, :])
                     return rc_sb

                 def norm_apply(st, hh, osb, rc_sb, p=p, hi_sb=hi_sb):
                     bc_ps = psmm.tile([64, 512], f32, tag="mm", name="mm")
                     nc.tensor.matmul(
                         bc_ps[:, :],
                         lhsT=ones_sb[64:65, 0:64],
                         rhs=rc_sb[64:65, :],
                         start=True,
                         stop=True,
                         tile_position=(64, 0),
                     )
                     dst = (
                         oT[p][0:64, 512 * st:512 * (st + 1)]
                         if hh == 0
                         else hi_sb[:, 512 * st:512 * (st + 1)]
                     )
                     nc.vector.scalar_tensor_tensor(
                         dst, osb[0:64, :], 1.0, bc_ps[:, :], MULT, MULT
                     )
                     if level >= 10 and hh == 1:
                         nc.sync.dma_start(
                             out=oT[p][64:128, 512 * st:512 * (st + 1)],
                             in_=hi_sb[:, 512 * st:512 * (st + 1)],
                         )

                 def normalize(st, hh, osb, p=p, hi_sb=hi_sb):
                     # osb: [65,512] SBUF f32 (level>=5) or PSUM tile (level<5)
                     rc_sb = recipp.tile([128, 512], bf16, tag="recip", name="recip")
                     with nc.allow_low_precision(reason="bf16 softmax denom"):
                         nc.vector.reciprocal(rc_sb[64:65, :], osb[64:65, :])
                     bc_ps = psmm.tile([64, 512], f32, tag="mm", name="mm")
                     if level >= 4:
                         nc.tensor.matmul(
                             bc_ps[:, :],
                             lhsT=ones_sb[64:65, 0:64],
                             rhs=rc_sb[64:65, :],
                             start=True,
                             stop=True,
                             tile_position=(64, 0),
                         )
                     else:
                         rc0_sb = recipp.tile([1, 512], bf16, tag="recip0", name="recip0")
                         nc.sync.dma_start(out=rc0_sb[0:1, :], in_=rc_sb[64:65, :])
                         nc.tensor.matmul(
                             bc_ps[:, :],
                             lhsT=ones_sb[0:1, 0:64],
                             rhs=rc0_sb[0:1, :],
                             start=True,
                             stop=True,
                         )
                     dst = (
                         oT[p][0:64, 512 * st:512 * (st + 1)]
                         if hh == 0
                         else hi_sb[:, 512 * st:512 * (st + 1)]
                     )
                     nc.vector.scalar_tensor_tensor(
                         dst, osb[0:64, :], 1.0, bc_ps[:, :], MULT, MULT
                     )
                     if level == 7 and hh == 1:
                         # per-supertile odd-head partition shift
                         nc.sync.dma_start(
                             out=oT[p][64:128, 512 * st:512 * (st + 1)],
                             in_=hi_sb[:, 512 * st:512 * (st + 1)],
                         )

                 def oproj_group(rt):
                     st_sb = ostagep.tile([128, 1024], f32, tag="ostage", name="ostage")
                     for o2 in range(2):
                         ps = psmm.tile([128, 512], f32, tag="mm", name="mm")
                         for c in range(4):
                             nc.tensor.matmul(
                                 ps[:],
                                 lhsT=oT[c][:, 128 * rt:128 * (rt + 1)],
                                 rhs=wo_sb[c][:, 512 * o2:512 * (o2 + 1)],
                                 start=(c == 0),
                                 stop=(c == 3),
                             )
                         if o2 == 0:
                             nc.vector.tensor_copy(
                                 st_sb[:, 512 * o2:512 * (o2 + 1)], ps[:]
                             )
                         else:
                             nc.scalar.copy(
                                 st_sb[:, 512 * o2:512 * (o2 + 1)], ps[:]
                             )
                     ([nc.sync, nc.scalar][rt % 2] if level >= 6 else nc.sync).dma_start(
                         out=out[128 * rt:128 * (rt + 1), :], in_=st_sb[:]
                     )

                 st_seen = {}
                 fill_iter = iter(pending)
                 nblocks = sum(4 * s + 4 for s in range(NST))
                 stride = max(1, nblocks // max(len(pending), 1))
                 blk = 0
                 for st in [s for s in range(NST) for _ in range(rep_attn)]:
                     out_ps = [
                         psout.tile([65, 512], f32, tag="o", name="o")
                         for _ in range(2)
                     ]
                     nkb = 4 * st + 4

                     def pv_full(kb, pr, first, out_ps=out_ps, nkb=nkb, p=p):
                         for hh in range(2):
                             base = 512 * hh
                             h = 2 * p + hh
                             nc.tensor.matmul(
                                 out_ps[hh][:, :],
                                 lhsT=v_rm[kb][:, 65 * h:65 * h + 65],
                                 rhs=pr[:, base:base + 512],
                                 start=first,
                                 stop=(kb == nkb - 1),
                             )

                     def pv_narrow(kb, r, qi0, pr, first, out_ps=out_ps, p=p):
                         for hh in range(2):
                             base = 512 * hh
                             h = 2 * p + hh
                             lhs = v_rm[kb][:, 65 * h:65 * h + 65]
                             if r < 0:
                                 nc.tensor.matmul(
                                     out_ps[hh][:, :],
                                     lhsT=lhs,
                                     rhs=pr[:, base:base + 512],
                                     start=first,
                                     stop=False,
                                     skip_group_check=True,
                                 )
                             else:
                                 nc.tensor.matmul(
                                     out_ps[hh][:, qi0:qi0 + 128],
                                     lhsT=lhs,
                                     rhs=pr[:, base + qi0:base + qi0 + 128],
                                     start=first,
                                     stop=True,
                                     skip_group_check=True,
                                 )
                                 if r < 3:
                                     nc.tensor.matmul(
                                         out_ps[hh][:, qi0 + 128:512],
                                         lhsT=lhs,
                                         rhs=pr[:, base + qi0 + 128:base + 512],
                                         start=first,
                                         stop=False,
                                         skip_group_check=True,
                                     )

                     pv_q = []
                     skew = 2 if level >= 9 else 1
                     for kb in range(nkb):
                         r = kb - 4 * st
                         qi0 = 128 * r if r > 0 else 0
                         first, last = (kb == 0), (kb == nkb - 1)
                         sc = pssc.tile([128, 1024], f32, tag="sc", name="sc")
                         if r >= 0 and level >= 3:
                             for hh in range(2):
                                 base = 512 * hh
                                 nc.tensor.matmul(
                                     sc[:, base + qi0:base + qi0 + 128],
                                     lhsT=eye_sb[:, :],
                                     rhs=maskb_sb[:, :],
                                     start=True,
                                     stop=False,
                                     skip_group_check=True,
                                 )
                         for hh in range(2):
                             base, lo = 512 * hh, 64 * hh
                             if r >= 0 and level >= 3:
                                 nc.tensor.matmul(
                                     sc[:, base + qi0:base + qi0 + 128],
                                     lhsT=kT[p][lo:lo + 64, 128 * kb:128 * (kb + 1)],
                                     rhs=qT[p][lo:lo + 64, 512 * st + qi0:512 * st + qi0 + 128],
                                     start=False,
                                     stop=True,
                                     tile_position=(lo, 0),
                                     skip_group_check=True,
                                 )
                                 if r < 3:
                                     nc.tensor.matmul(
                                         sc[:, base + qi0 + 128:base + 512],
                                         lhsT=kT[p][lo:lo + 64, 128 * kb:128 * (kb + 1)],
                                         rhs=qT[p][lo:lo + 64, 512 * st + qi0 + 128:512 * (st + 1)],
                                         start=True,
                                         stop=True,
                                         tile_position=(lo, 0),
                                         skip_group_check=True,
                                     )
                             else:
                                 nc.tensor.matmul(
                                     sc[:, base + qi0:base + 512],
                                     lhsT=kT[p][lo:lo + 64, 128 * kb:128 * (kb + 1)],
                                     rhs=qT[p][lo:lo + 64, 512 * st + qi0:512 * (st + 1)],
                                     start=True,
                                     stop=True,
                                     tile_position=(lo, 0),
                                 )
                         if r >= 0 and level < 3:
                             for hh in range(2):
                                 base = 512 * hh
                                 nc.vector.tensor_add(
                                     sc[:, base + qi0:base + qi0 + 128],
                                     sc[:, base + qi0:base + qi0 + 128],
                                     mask_sb[:, :],
                                 )
                         pr = probsp.tile([128, 1024], bf16, tag="pr", name="pr")
                         if qi0 == 0:
                             nc.scalar.activation(pr[:], sc[:], AF.Exp)
                         else:
                             pr_v = pr.rearrange("p (h q) -> p h q", h=2)
                             sc_v = sc.rearrange("p (h q) -> p h q", h=2)
                             if level < 2:
                                 nc.vector.memset(pr_v[:, :, 0:qi0], 0.0)
                             nc.scalar.activation(
                                 pr_v[:, :, qi0:512], sc_v[:, :, qi0:512], AF.Exp
                             )

                         if level >= 2:
                             this_pv = (
                                 lambda f=pv_narrow, kb=kb, r=r, qi0=qi0, pr=pr,
                                 first=first: f(kb, r, qi0, pr, first)
                             )
                         else:
                             this_pv = (
                                 lambda f=pv_full, kb=kb, pr=pr, first=first:
                                 f(kb, pr, first)
                             )
                         if level >= 1:
                             pv_q.append(this_pv)
                             if len(pv_q) > skew:
                                 pv_q.pop(0)()
                         else:
                             this_pv()

                         if level >= 10:
                             if normq and kb >= normq[0][0]:
                                 normq.pop(0)[2]()
                         else:
                             if level >= 9:
                                 popper = kb >= 1
                             elif level == 7 or level == 8:
                                 popper = kb % 2 == 1 or p == NP - 1
                             else:
                                 popper = kb % 2 == 1
                             if level >= 5 and popper and normq:
                                 normq.pop(0)()
                         blk += 1
                         if blk % stride == 0:
                             g = next(fill_iter, None)
                             if g is not None:
                                 g()
                     if level >= 1:
                         for g in pv_q:
                             g()
                         pv_q = []
                     if level >= 9:
                         stage_bs = []
                         for hh in range(2):
                             o_sb = recipp.tile([65, 512], f32, tag="osb", name="osb")
                             nc.vector.tensor_copy(o_sb[:, :], out_ps[hh][:, :])
                             holder = {}

                             def stage_a(f=norm_recip, osb=o_sb, holder=holder):
                                 holder["rc"] = f(osb)

                             def stage_b(f=norm_apply, st=st, hh=hh, osb=o_sb,
                                         holder=holder):
                                 f(st, hh, osb, holder["rc"])

                             if level >= 10:
                                 normq.append((1, "a", stage_a))
                                 stage_bs.append((5, "b", stage_b))
                             else:
                                 normq.append(stage_a)
                                 stage_bs.append(stage_b)
                         normq.extend(stage_bs)
                     elif level >= 5:
                         for hh in range(2):
                             o_sb = recipp.tile([65, 512], f32, tag="osb", name="osb")
                             nc.vector.tensor_copy(o_sb[:, :], out_ps[hh][:, :])
                             normq.append(
                                 lambda f=normalize, st=st, hh=hh, osb=o_sb:
                                 f(st, hh, osb)
                             )
                         st_seen[st] = st_seen.get(st, 0) + 1
                         if (level in (7, 8) and p == NP - 1
                                 and st_seen[st] == rep_attn):
                             for rt in range(4 * st, 4 * st + 4):
                                 normq.append(
                                     lambda f=oproj_group, rt=rt: f(rt)
                                 )
                     else:
                         for hh in range(2):
                             o_sb = recipp.tile([65, 512], f32, tag="osb", name="osb")
                             nc.vector.tensor_copy(o_sb[:, :], out_ps[hh][:, :])
                             normalize(st, hh, o_sb)
                 for g in fill_iter:
                     g()
                 if level in (7, 8) or level >= 10:
                     pass  # per-supertile shifts emitted inside normalize
                 elif level >= 5:
                     def shift(hi_sb=hi_sb, p=p):
                         nc.sync.dma_start(out=oT[p][64:128, :], in_=hi_sb[:, :])
                     normq.append(shift)
                 else:
                     nc.sync.dma_start(out=oT[p][64:128, :], in_=hi_sb[:, :])

             def oproj_rt(rt):
                 st_sb = ostagep.tile([128, 1024], f32, tag="ostage", name="ostage")
                 for o2 in range(2):
                     ps = psmm.tile([128, 512], f32, tag="mm", name="mm")
                     for c in range(4):
                         nc.tensor.matmul(
                             ps[:],
                             lhsT=oT[c][:, 128 * rt:128 * (rt + 1)],
                             rhs=wo_sb[c][:, 512 * o2:512 * (o2 + 1)],
                             start=(c == 0),
                             stop=(c == 3),
                         )
                     if o2 == 0:
                         nc.vector.tensor_copy(
                             st_sb[:, 512 * o2:512 * (o2 + 1)], ps[:]
                         )
                     else:
                         nc.scalar.copy(
                             st_sb[:, 512 * o2:512 * (o2 + 1)], ps[:]
                         )
                 ([nc.sync, nc.scalar][rt % 2] if level >= 6 else nc.sync).dma_start(
                     out=out[128 * rt:128 * (rt + 1), :], in_=st_sb[:]
                 )

             if level >= 10:
                 # tail: start recips, hide the final normalize chains under
                 # oproj rows that only need supertiles 0-2, finish with the
                 # last supertile's rows
                 while normq and normq[0][1] == "a":
                     normq.pop(0)[2]()
                 for rt in range(12):
                     oproj_rt(rt)
                 while normq:
                     normq.pop(0)[2]()
                 for rt in range(12, NRT):
                     oproj_rt(rt)
                 for _orep in range(rep_oproj - 1):
                     for rt in range(NRT):
                         oproj_rt(rt)
             else:
                 while normq:
                     normq.pop(0)()
                 for _orep in range(rep_oproj - 1 if level in (7, 8) else rep_oproj):
                     for rt in range(NRT):
                         oproj_rt(rt)
    nc.finalize()
    return nc


def get_program():
    if "nc" not in _PROG_CACHE:
        _PROG_CACHE["nc"] = build_program()
    return _PROG_CACHE["nc"]


def make_in_maps(x, w_qkv, w_out):
    import ml_dtypes

    bf = ml_dtypes.bfloat16
    x = np.asarray(x, dtype=np.float32)
    w_qkv = np.asarray(w_qkv, dtype=np.float32)
    w_out = np.asarray(w_out, dtype=np.float32)
    scale = float(D) ** -0.5
    p_idx = np.arange(128)[:, None]
    j_idx = np.arange(128)[None, :]
    maskf = np.where(p_idx > j_idx, NEG, 0.0).astype(np.float32)
    in_maps = []
    for c in range(NCORES):
        b, hh = c // 2, c % 2
        q0 = CQ * hh
        wq = (w_qkv[:, q0:q0 + CQ] * scale).astype(bf)
        wk = w_qkv[:, DM + q0:DM + q0 + CQ].astype(bf)
        in_maps.append(
            {
                "xt": np.ascontiguousarray(x[b].T).astype(bf),
                "wqk": np.concatenate([wq, wk], axis=1),
                "wv": w_qkv[:, 2 * DM + q0:2 * DM + q0 + CQ].astype(bf),
                "wo": w_out[q0:q0 + CQ, :].astype(bf),
                "mask": maskf,
                "maskb": maskf.astype(bf),
                "eye": np.eye(128, dtype=bf),
            }
        )
    return in_maps


def gather(results):
    outs = [np.asarray(results[c]["out"], dtype=np.float32) for c in range(NCORES)]
    return np.stack([outs[2 * b] + outs[2 * b + 1] for b in range(B)], axis=0)


def kernel(x, w_qkv, w_out):
    from concourse.bass_utils import run_bass_kernel_spmd

    nc = get_program()
    in_maps = make_in_maps(x, w_qkv, w_out)
    res = run_bass_kernel_spmd(nc, in_maps, list(range(NCORES)))
    return gather(res.results)


# revision 7
# speedup vs baseline: 1.1009x; 1.0359x over previous
"""Trainium2 Bass kernel for causal MultiHeadAttention.

Problem: B=4, S=2048, H=16, D=64, DM=1024, fp32 I/O.
  qkv = x @ w_qkv ; causal softmax attention per head ; out = attn @ w_out

Sharding (8 cores): 4-way batch x 2-way heads. Core c handles batch c//2 and
heads (c%2)*8 .. +8; each core computes a partial out-projection (its 512
attention channels x full w_out row-slice) and the host sums the two
head-half partials per batch while unsharding.

Per-core dataflow (bf16 matmul inputs, fp32 PSUM): staged Q/K/V projections
feed a block-causal attention stream (key blocks of 128 x query supertiles
of 512, two heads packed in the PE array), normalization runs as deferred
closures inside the next supertile's block stream, and the out-projection
tail hides the final normalize chains. The optimization history is encoded
as cumulative levels (LEVEL=12 enables the best verified set):

level 0: session-1 baseline structure
level 1: + PV software-pipelined one block behind scores (skew)
level 2: + PV causally narrowed (no pr memsets)
level 3: + wedge mask pre-written to PSUM by PE identity matmul
level 4: + recip row at partition 64 (no rc0 shift DMA)
level 5: + deferred normalization via closure queue (o_sb evict at st end)
level 6: + input/output DMA spread over sync+scalar queues kt-ordered
level 7: + out-projection groups interleaved into pair-3's attention via the
         closure queue (per-supertile odd-head shift DMAs)
level 8: + gpsimd as third input/output DMA queue
level 9: + PV skew 2, normalize chain split into finer closures (recip
         popped a block before broadcast+multiply), projection evictions
         alternating DVE/Act  (on top of level 6; levels 7-8 were
         HW-neutral/negative and are skipped when level >= 9)
level 10: + stage_b (broadcast+multiply) pops gated to kb>=5 so the DVE
         recip chain always completes first; per-supertile odd-head shift
         DMAs; ordered tail flush (recips -> oproj rt 0-11 -> final
         normalizes -> oproj rt 12-15) so the out-projection hides the
         last normalization chains
level 11: + attention-output PSUM evictions on the scalar engine so the
         DVE recip chain is never queued behind them
level 12: + PV skew 3 (probs pool 6 bufs) and single narrowed PV matmul per
         head per block (no stop-flag A/B split; relies on instruction-level
         deps, skip_group_check)  — on top of level 10, skipping 11
"""

import numpy as np

B, S, H, D = 4, 2048, 16, 64
DM = H * D          # 1024
NCORES = 8
HPC = H // 2        # 8 heads per core
CQ = HPC * D        # 512 channels per core
NEG = -1.0e9

_PROG_CACHE = {}
LEVEL = 12


def build_program(rep_qkv=1, rep_attn=1, rep_oproj=1, rep_all=1, level=None):
    import concourse.mybir as mybir
    import concourse.tile as tile
    from concourse import bacc

    if level is None:
        level = LEVEL
    dt = mybir.dt
    f32 = dt.float32
    bf16 = dt.bfloat16
    AF = mybir.ActivationFunctionType
    MULT = mybir.AluOpType.mult

    nc = bacc.Bacc(None)
    xt = nc.declare_dram_parameter("xt", [DM, S], bf16, isOutput=False)
    wqk = nc.declare_dram_parameter("wqk", [DM, 2 * CQ], bf16, isOutput=False)
    wv = nc.declare_dram_parameter("wv", [DM, CQ], bf16, isOutput=False)
    wo = nc.declare_dram_parameter("wo", [CQ, DM], bf16, isOutput=False)
    mask = nc.declare_dram_parameter("mask", [128, 128], f32, isOutput=False)
    maskb = nc.declare_dram_parameter("maskb", [128, 128], bf16, isOutput=False)
    eye = nc.declare_dram_parameter("eye", [128, 128], bf16, isOutput=False)
    out = nc.declare_dram_parameter("out", [S, DM], f32, isOutput=True)

    KT = DM // 128
    NRT = S // 128
    NRC = S // 512
    NP = HPC // 2
    NST = S // 512

    with tile.TileContext(nc) as tc:
        with (
            tc.tile_pool(name="persist", bufs=1) as pp,
            tc.tile_pool(name="probs", bufs=6) as probsp,
            tc.tile_pool(name="recip", bufs=4) as recipp,
            tc.tile_pool(name="ostage", bufs=3) as ostagep,
            tc.tile_pool(name="psmm", bufs=2, space="PSUM") as psmm,
            tc.tile_pool(name="pssc", bufs=2, space="PSUM") as pssc,
            tc.tile_pool(name="psout", bufs=2, space="PSUM") as psout,
        ):
            # ---- load inputs to SBUF ----
            if level >= 6:
                queues = [nc.sync, nc.scalar, nc.gpsimd][:3 if level == 8 else 2]
                qcnt = [0]

                def load(t, src):
                    queues[qcnt[0] % len(queues)].dma_start(out=t[:], in_=src)
                    qcnt[0] += 1
            else:
                def load(t, src, q=[0]):
                    (nc.sync if q[0] % 3 == 0 else nc.scalar).dma_start(
                        out=t[:], in_=src)
                    q[0] += 1

            mask_sb = pp.tile([128, 128], f32, tag="mask", name="mask")
            load(mask_sb, mask[:, :])
            maskb_sb = pp.tile([128, 128], bf16, tag="maskb", name="maskb")
            load(maskb_sb, maskb[:, :])
            eye_sb = pp.tile([128, 128], bf16, tag="eye", name="eye")
            load(eye_sb, eye[:, :])
            xt_sb = []
            wqk_sb = []
            wv_sb = []
            for i in range(KT):
                t = pp.tile([128, S], bf16, tag=f"xt{i}", name=f"xt{i}")
                load(t, xt[128 * i:128 * (i + 1), :])
                xt_sb.append(t)
                t = pp.tile([128, 2 * CQ], bf16, tag=f"wqk{i}", name=f"wqk{i}")
                load(t, wqk[128 * i:128 * (i + 1), :])
                wqk_sb.append(t)
            for i in range(KT):
                t = pp.tile([128, CQ], bf16, tag=f"wv{i}", name=f"wv{i}")
                load(t, wv[128 * i:128 * (i + 1), :])
                wv_sb.append(t)
            wo_sb = []
            for c in range(CQ // 128):
                t = pp.tile([128, DM], bf16, tag=f"wo{c}", name=f"wo{c}")
                load(t, wo[128 * c:128 * (c + 1), :])
                wo_sb.append(t)
            ones_sb = pp.tile([128, 64], bf16, tag="ones", name="ones")
            nc.vector.memset(ones_sb[:, :], 1.0)

            qT = [pp.tile([128, S], bf16, tag=f"qT{p}", name=f"qT{p}") for p in range(NP)]
            kT = [pp.tile([128, S], bf16, tag=f"kT{p}", name=f"kT{p}") for p in range(NP)]
            v_rm = [pp.tile([128, HPC * 65], bf16, tag=f"v{rt}", name=f"v{rt}") for rt in range(NRT)]
            oT = [pp.tile([128, S], bf16, tag=f"oT{p}", name=f"oT{p}") for p in range(NP)]

            def v_group(rt):
                v_view = v_rm[rt].rearrange("p (h c) -> p h c", c=65)
                nc.vector.memset(v_view[:, :, 64:65], 1.0)
                ps = psmm.tile([128, 512], f32, tag="mm", name="mm")
                for kt in range(KT):
                    nc.tensor.matmul(
                        ps[:],
                        lhsT=xt_sb[kt][:, 128 * rt:128 * (rt + 1)],
                        rhs=wv_sb[kt][:],
                        start=(kt == 0),
                        stop=(kt == KT - 1),
                    )
                if level >= 9 and rt % 2 == 1:
                    nc.scalar.copy(
                        v_view[:, :, 0:64], ps.rearrange("p (h c) -> p h c", c=64)
                    )
                else:
                    nc.vector.tensor_copy(
                        v_view[:, :, 0:64], ps.rearrange("p (h c) -> p h c", c=64)
                    )

            for _arep in range(rep_all):

             def qk_group(p, ct, rc):
                 dst = qT[p] if ct < NP else kT[p]
                 ps = psmm.tile([128, 512], f32, tag="mm", name="mm")
                 for kt in range(KT):
                     nc.tensor.matmul(
                         ps[:],
                         lhsT=wqk_sb[kt][:, 128 * ct:128 * (ct + 1)],
                         rhs=xt_sb[kt][:, 512 * rc:512 * (rc + 1)],
                         start=(kt == 0),
                         stop=(kt == KT - 1),
                     )
                 if level >= 9 and rc % 2 == 1:
                     nc.scalar.copy(dst[:, 512 * rc:512 * (rc + 1)], ps[:])
                 else:
                     nc.vector.tensor_copy(dst[:, 512 * rc:512 * (rc + 1)], ps[:])

             qk_groups = lambda p: [
                 (lambda a=p, b=ct, c=rc: qk_group(a, b, c))
                 for ct in (p, NP + p) for rc in range(NRC)
             ]

             for _qrep in range(rep_qkv):
                 for g in qk_groups(0):
                     g()
             for rt in range(4):
                 v_group(rt)

             normq = []

             for p in range(NP):
                 pending = []
                 if p == 0:
                     pending += [lambda a=rt: v_group(a) for rt in range(4, NRT)]
                 pending += qk_groups(p + 1) if p + 1 < NP else []
                 if rep_qkv > 1:
                     for _qrep in range(rep_qkv - 1):
                         for g in qk_groups(p):
                             g()
                     if p == 0:
                         for rt in range(4, NRT):
                             v_group(rt)
                         pending = qk_groups(p + 1) if p + 1 < NP else []

                 hi_sb = recipp.tile([64, S], bf16, tag="hi", name="hi")

                 def norm_recip(osb):
                     rc_sb = recipp.tile([128, 512], bf16, tag="recip", name="recip")
                     with nc.allow_low_precision(reason="bf16 softmax denom"):
                         nc.vector.reciprocal(rc_sb[64:65, :], osb[64:65, :])
                     return rc_sb

                 def norm_apply(st, hh, osb, rc_sb, p=p, hi_sb=hi_sb):
                     bc_ps = psmm.tile([64, 512], f32, tag="mm", name="mm")
                     nc.tensor.matmul(
                         bc_ps[:, :],
                         lhsT=ones_sb[64:65, 0:64],
                         rhs=rc_sb[64:65, :],
                         start=True,
                         stop=True,
                         tile_position=(64, 0),
                     )
                     dst = (
                         oT[p][0:64, 512 * st:512 * (st + 1)]
                         if hh == 0
                         else hi_sb[:, 512 * st:512 * (st + 1)]
                     )
                     nc.vector.scalar_tensor_tensor(
                         dst, osb[0:64, :], 1.0, bc_ps[:, :], MULT, MULT
                     )
                     if level >= 10 and hh == 1:
                         nc.sync.dma_start(
                             out=oT[p][64:128, 512 * st:512 * (st + 1)],
                             in_=hi_sb[:, 512 * st:512 * (st + 1)],
                         )

                 def normalize(st, hh, osb, p=p, hi_sb=hi_sb):
                     # osb: [65,512] SBUF f32 (level>=5) or PSUM tile (level<5)
                     rc_sb = recipp.tile([128, 512], bf16, tag="recip", name="recip")
                     with nc.allow_low_precision(reason="bf16 softmax denom"):
                         nc.vector.reciprocal(rc_sb[64:65, :], osb[64:65, :])
                     bc_ps = psmm.tile([64, 512], f32, tag="mm", name="mm")
                     if level >= 4:
                         nc.tensor.matmul(
                             bc_ps[:, :],
                             lhsT=ones_sb[64:65, 0:64],
                             rhs=rc_sb[64:65, :],
                             start=True,
                             stop=True,
                             tile_position=(64, 0),
                         )
                     else:
                         rc0_sb = recipp.tile([1, 512], bf16, tag="recip0", name="recip0")
                         nc.sync.dma_start(out=rc0_sb[0:1, :], in_=rc_sb[64:65, :])
                         nc.tensor.matmul(
                             bc_ps[:, :],
                             lhsT=ones_sb[0:1, 0:64],
                             rhs=rc0_sb[0:1, :],
                             start=True,
                             stop=True,
                         )
                     dst = (
                         oT[p][0:64, 512 * st:512 * (st + 1)]
                         if hh == 0
                         else hi_sb[:, 512 * st:512 * (st + 1)]
                     )
                     nc.vector.scalar_tensor_tensor(
                         dst, osb[0:64, :], 1.0, bc_ps[:, :], MULT, MULT
                     )
                     if level == 7 and hh == 1:
                         # per-supertile odd-head partition shift
                         nc.sync.dma_start(
                             out=oT[p][64:128, 512 * st:512 * (st + 1)],
                             in_=hi_sb[:, 512 * st:512 * (st + 1)],
                         )

                 def oproj_group(rt):
                     st_sb = ostagep.tile([128, 1024], f32, tag="ostage", name="ostage")
                     for o2 in range(2):
                         ps = psmm.tile([128, 512], f32, tag="mm", name="mm")
                         for c in range(4):
                             nc.tensor.matmul(
                                 ps[:],
                                 lhsT=oT[c][:, 128 * rt:128 * (rt + 1)],
                                 rhs=wo_sb[c][:, 512 * o2:512 * (o2 + 1)],
                                 start=(c == 0),
                                 stop=(c == 3),
                             )
                         if o2 == 0:
                             nc.vector.tensor_copy(
                                 st_sb[:, 512 * o2:512 * (o2 + 1)], ps[:]
                             )
                         else:
                             nc.scalar.copy(
                                 st_sb[:, 512 * o2:512 * (o2 + 1)], ps[:]
                             )
                     ([nc.sync, nc.scalar][rt % 2] if level >= 6 else nc.sync).dma_start(
                         out=out[128 * rt:128 * (rt + 1), :], in_=st_sb[:]
                     )

                 st_seen = {}
                 fill_iter = iter(pending)
                 nblocks = sum(4 * s + 4 for s in range(NST))
                 stride = max(1, nblocks // max(len(pending), 1))
                 blk = 0
                 for st in [s for s in range(NST) for _ in range(rep_attn)]:
                     out_ps = [
                         psout.tile([65, 512], f32, tag="o", name="o")
                         for _ in range(2)
                     ]
                     nkb = 4 * st + 4

                     def pv_full(kb, pr, first, out_ps=out_ps, nkb=nkb, p=p):
                         for hh in range(2):
                             base = 512 * hh
                             h = 2 * p + hh
                             nc.tensor.matmul(
                                 out_ps[hh][:, :],
                                 lhsT=v_rm[kb][:, 65 * h:65 * h + 65],
                                 rhs=pr[:, base:base + 512],
                                 start=first,
                                 stop=(kb == nkb - 1),
                             )

                     def pv_narrow(kb, r, qi0, pr, first, out_ps=out_ps, p=p,
                                   nkb=nkb):
                         for hh in range(2):
                             base = 512 * hh
                             h = 2 * p + hh
                             lhs = v_rm[kb][:, 65 * h:65 * h + 65]
                             if level >= 12:
                                 nc.tensor.matmul(
                                     out_ps[hh][:, qi0:512],
                                     lhsT=lhs,
                                     rhs=pr[:, base + qi0:base + 512],
                                     start=first,
                                     stop=(kb == nkb - 1),
                                     skip_group_check=True,
                                 )
                                 continue
                             if r < 0:
                                 nc.tensor.matmul(
                                     out_ps[hh][:, :],
                                     lhsT=lhs,
                                     rhs=pr[:, base:base + 512],
                                     start=first,
                                     stop=False,
                                     skip_group_check=True,
                                 )
                             else:
                                 nc.tensor.matmul(
                                     out_ps[hh][:, qi0:qi0 + 128],
                                     lhsT=lhs,
                                     rhs=pr[:, base + qi0:base + qi0 + 128],
                                     start=first,
                                     stop=True,
                                     skip_group_check=True,
                                 )
                                 if r < 3:
                                     nc.tensor.matmul(
                                         out_ps[hh][:, qi0 + 128:512],
                                         lhsT=lhs,
                                         rhs=pr[:, base + qi0 + 128:base + 512],
                                         start=first,
                                         stop=False,
                                         skip_group_check=True,
                                     )

                     pv_q = []
                     skew = 3 if level >= 12 else (2 if level >= 9 else 1)
                     for kb in range(nkb):
                         r = kb - 4 * st
                         qi0 = 128 * r if r > 0 else 0
                         first, last = (kb == 0), (kb == nkb - 1)
                         sc = pssc.tile([128, 1024], f32, tag="sc", name="sc")
                         if r >= 0 and level >= 3:
                             for hh in range(2):
                                 base = 512 * hh
                                 nc.tensor.matmul(
                                     sc[:, base + qi0:base + qi0 + 128],
                                     lhsT=eye_sb[:, :],
                                     rhs=maskb_sb[:, :],
                                     start=True,
                                     stop=False,
                                     skip_group_check=True,
                                 )
                         for hh in range(2):
                             base, lo = 512 * hh, 64 * hh
                             if r >= 0 and level >= 3:
                                 nc.tensor.matmul(
                                     sc[:, base + qi0:base + qi0 + 128],
                                     lhsT=kT[p][lo:lo + 64, 128 * kb:128 * (kb + 1)],
                                     rhs=qT[p][lo:lo + 64, 512 * st + qi0:512 * st + qi0 + 128],
                                     start=False,
                                     stop=True,
                                     tile_position=(lo, 0),
                                     skip_group_check=True,
                                 )
                                 if r < 3:
                                     nc.tensor.matmul(
                                         sc[:, base + qi0 + 128:base + 512],
                                         lhsT=kT[p][lo:lo + 64, 128 * kb:128 * (kb + 1)],
                                         rhs=qT[p][lo:lo + 64, 512 * st + qi0 + 128:512 * (st + 1)],
                                         start=True,
                                         stop=True,
                                         tile_position=(lo, 0),
                                         skip_group_check=True,
                                     )
                             else:
                                 nc.tensor.matmul(
                                     sc[:, base + qi0:base + 512],
                                     lhsT=kT[p][lo:lo + 64, 128 * kb:128 * (kb + 1)],
                                     rhs=qT[p][lo:lo + 64, 512 * st + qi0:512 * (st + 1)],
                                     start=True,
                                     stop=True,
                                     tile_position=(lo, 0),
                                 )
                         if r >= 0 and level < 3:
                             for hh in range(2):
                                 base = 512 * hh
                                 nc.vector.tensor_add(
                                     sc[:, base + qi0:base + qi0 + 128],
                                     sc[:, base + qi0:base + qi0 + 128],
                                     mask_sb[:, :],
                                 )
                         pr = probsp.tile([128, 1024], bf16, tag="pr", name="pr")
                         if qi0 == 0:
                             nc.scalar.activation(pr[:], sc[:], AF.Exp)
                         else:
                             pr_v = pr.rearrange("p (h q) -> p h q", h=2)
                             sc_v = sc.rearrange("p (h q) -> p h q", h=2)
                             if level < 2:
                                 nc.vector.memset(pr_v[:, :, 0:qi0], 0.0)
                             nc.scalar.activation(
                                 pr_v[:, :, qi0:512], sc_v[:, :, qi0:512], AF.Exp
                             )

                         if level >= 2:
                             this_pv = (
                                 lambda f=pv_narrow, kb=kb, r=r, qi0=qi0, pr=pr,
                                 first=first: f(kb, r, qi0, pr, first)
                             )
                         else:
                             this_pv = (
                                 lambda f=pv_full, kb=kb, pr=pr, first=first:
                                 f(kb, pr, first)
                             )
                         if level >= 1:
                             pv_q.append(this_pv)
                             if len(pv_q) > skew:
                                 pv_q.pop(0)()
                         else:
                             this_pv()

                         if level >= 10:
                             if normq and kb >= normq[0][0]:
                                 normq.pop(0)[2]()
                         else:
                             if level >= 9:
                                 popper = kb >= 1
                             elif level == 7 or level == 8:
                                 popper = kb % 2 == 1 or p == NP - 1
                             else:
                                 popper = kb % 2 == 1
                             if level >= 5 and popper and normq:
                                 normq.pop(0)()
                         blk += 1
                         if blk % stride == 0:
                             g = next(fill_iter, None)
                             if g is not None:
                                 g()
                     if level >= 1:
                         for g in pv_q:
                             g()
                         pv_q = []
                     if level >= 9:
                         stage_bs = []
                         for hh in range(2):
                             o_sb = recipp.tile([65, 512], f32, tag="osb", name="osb")
                             if level == 11 and hh == 1:
                                 nc.scalar.copy(o_sb[:, :], out_ps[hh][:, :])
                             else:
                                 nc.vector.tensor_copy(o_sb[:, :], out_ps[hh][:, :])
                             holder = {}

                             def stage_a(f=norm_recip, osb=o_sb, holder=holder):
                                 holder["rc"] = f(osb)

                             def stage_b(f=norm_apply, st=st, hh=hh, osb=o_sb,
                                         holder=holder):
                                 f(st, hh, osb, holder["rc"])

                             if level >= 10:
                                 normq.append((1, "a", stage_a))
                                 stage_bs.append((5, "b", stage_b))
                             else:
                                 normq.append(stage_a)
                                 stage_bs.append(stage_b)
                         normq.extend(stage_bs)
                     elif level >= 5:
                         for hh in range(2):
                             o_sb = recipp.tile([65, 512], f32, tag="osb", name="osb")
                             nc.vector.tensor_copy(o_sb[:, :], out_ps[hh][:, :])
                             normq.append(
                                 lambda f=normalize, st=st, hh=hh, osb=o_sb:
                                 f(st, hh, osb)
                             )
                         st_seen[st] = st_seen.get(st, 0) + 1
                         if (level in (7, 8) and p == NP - 1
                                 and st_seen[st] == rep_attn):
                             for rt in range(4 * st, 4 * st + 4):
                                 normq.append(
                                     lambda f=oproj_group, rt=rt: f(rt)
                                 )
                     else:
                         for hh in range(2):
                             o_sb = recipp.tile([65, 512], f32, tag="osb", name="osb")
                             nc.vector.tensor_copy(o_sb[:, :], out_ps[hh][:, :])
                             normalize(st, hh, o_sb)
                 for g in fill_iter:
                     g()
                 if level in (7, 8) or level >= 10:
                     pass  # per-supertile shifts emitted inside normalize
                 elif level >= 5:
                     def shift(hi_sb=hi_sb, p=p):
                         nc.sync.dma_start(out=oT[p][64:128, :], in_=hi_sb[:, :])
                     normq.append(shift)
                 else:
                     nc.sync.dma_start(out=oT[p][64:128, :], in_=hi_sb[:, :])

             def oproj_rt(rt):
                 st_sb = ostagep.tile([128, 1024], f32, tag="ostage", name="ostage")
                 for o2 in range(2):
                     ps = psmm.tile([128, 512], f32, tag="mm", name="mm")
                     for c in range(4):
                         nc.tensor.matmul(
                             ps[:],
                             lhsT=oT[c][:, 128 * rt:128 * (rt + 1)],
                             rhs=wo_sb[c][:, 512 * o2:512 * (o2 + 1)],
                             start=(c == 0),
                             stop=(c == 3),
                         )
                     if o2 == 0:
                         nc.vector.tensor_copy(
                             st_sb[:, 512 * o2:512 * (o2 + 1)], ps[:]
                         )
                     else:
                         nc.scalar.copy(
                             st_sb[:, 512 * o2:512 * (o2 + 1)], ps[:]
                         )
                 ([nc.sync, nc.scalar][rt % 2] if level >= 6 else nc.sync).dma_start(
                     out=out[128 * rt:128 * (rt + 1), :], in_=st_sb[:]
                 )

             if level >= 10:
                 # tail: start recips, hide the final normalize chains under
                 # oproj rows that only need supertiles 0-2, finish with the
                 # last supertile's rows
                 while normq and normq[0][1] == "a":
                     normq.pop(0)[2]()
                 for rt in range(12):
                     oproj_rt(rt)
                 while normq:
                     normq.pop(0)[2]()
                 for rt in range(12, NRT):
                     oproj_rt(rt)
                 for _orep in range(rep_oproj - 1):
                     for rt in range(NRT):
                         oproj_rt(rt)
             else:
                 while normq:
                     normq.pop(0)()
                 for _orep in range(rep_oproj - 1 if level in (7, 8) else rep_oproj):
                     for rt in range(NRT):
                         oproj_rt(rt)
    nc.finalize()
    return nc


def get_program():
    if "nc" not in _PROG_CACHE:
        _PROG_CACHE["nc"] = build_program()
    return _PROG_CACHE["nc"]


def make_in_maps(x, w_qkv, w_out):
    import ml_dtypes

    bf = ml_dtypes.bfloat16
    x = np.asarray(x, dtype=np.float32)
    w_qkv = np.asarray(w_qkv, dtype=np.float32)
    w_out = np.asarray(w_out, dtype=np.float32)
    scale = float(D) ** -0.5
    p_idx = np.arange(128)[:, None]
    j_idx = np.arange(128)[None, :]
    maskf = np.where(p_idx > j_idx, NEG, 0.0).astype(np.float32)
    in_maps = []
    for c in range(NCORES):
        b, hh = c // 2, c % 2
        q0 = CQ * hh
        wq = (w_qkv[:, q0:q0 + CQ] * scale).astype(bf)
        wk = w_qkv[:, DM + q0:DM + q0 + CQ].astype(bf)
        in_maps.append(
            {
                "xt": np.ascontiguousarray(x[b].T).astype(bf),
                "wqk": np.concatenate([wq, wk], axis=1),
                "wv": w_qkv[:, 2 * DM + q0:2 * DM + q0 + CQ].astype(bf),
                "wo": w_out[q0:q0 + CQ, :].astype(bf),
                "mask": maskf,
                "maskb": maskf.astype(bf),
                "eye": np.eye(128, dtype=bf),
            }
        )
    return in_maps


def gather(results):
    outs = [np.asarray(results[c]["out"], dtype=np.float32) for c in range(NCORES)]
    return np.stack([outs[2 * b] + outs[2 * b + 1] for b in range(B)], axis=0)


def kernel(x, w_qkv, w_out):
    from concourse.bass_utils import run_bass_kernel_spmd

    nc = get_program()
    in_maps = make_in_maps(x, w_qkv, w_out)
    res = run_bass_kernel_spmd(nc, in_maps, list(range(NCORES)))
    return gather(res.results)
